# revision 31
# baseline (speedup 1.0000x reference)
# Trainium2 Bass kernel for MemEffAttentionRope (B=2, N=2048, C=1024, H=16, D=64).
#
# Sharding: tensor-parallel over heads - each of the 8 cores owns 2 heads for
# both batches. Per core: qkv projection (only its heads' weight rows), qk
# layernorm + rope, full attention for its 4 (batch, head) pairs, then an
# AllToAll (split per batch, so batch 0's collective and output projection
# hide under batch 1's attention) reshards head-major attention output to
# token-major; each core computes the output projection for 256 tokens of each
# batch. Host concatenates the 8 token slices.
#
# All matmul operands are bf16 (full-rate PE, half the SBUF/DMA/collective
# bytes, 2x DVE element-wise); PSUM accumulation, LN stats and the softmax
# denominator reciprocal stay fp32. LayerNorm is fused into stage 1: bn_stats
# per tile, the even/odd 6-tuples aggregated with a few batched strided DVE
# ops per 4-tile group, and the normalize itself runs on the scalar engine
# (Identity activation, per-partition scale=rstd bias=-mean*rstd) straight out
# of PSUM into bf16 SBUF. Softmax skips the max-subtraction: q,k are
# layernormed so |q.k|*scale <= ~10 and exp stays well inside fp32 range. The
# denominator comes from a ones-column appended to the PV stationary operand
# (output partition 64); it is broadcast across partitions via a DRAM bounce.
import sys

sys.path.insert(0, "/opt/trn_rl_repo")

import numpy as np
import ml_dtypes

B, N, C = 2, 2048, 1024
H, D = 16, 64
T = B * N
NCORES = 8
HPC = H // NCORES  # heads per core = 2
EPS = 1e-5
SCALE = D ** -0.5
TSLICE = T // NCORES  # tokens per core after reshard = 512

_BUILT = {}


def _build(skip_affine):
    key = ("nc", skip_affine)
    if key in _BUILT:
        return _BUILT[key]

    import concourse.bass as bass
    import concourse.mybir as mybir
    import concourse.tile as tile
    from concourse import bacc
    from concourse.masks import make_identity

    f32 = mybir.dt.float32
    bf16 = mybir.dt.bfloat16
    AF = mybir.ActivationFunctionType
    ALU = mybir.AluOpType

    nc = bacc.Bacc(None, target_bir_lowering=False, debug=False)

    xT = nc.dram_tensor("xT", [C, T], bf16, kind="ExternalInput")
    wqk = nc.dram_tensor("wqk", [C, 3 * HPC * D], bf16, kind="ExternalInput")
    pwT = nc.dram_tensor("pwT", [C, C], bf16, kind="ExternalInput")
    cos4 = nc.dram_tensor("cos4", [N, 256], bf16, kind="ExternalInput")
    sin4 = nc.dram_tensor("sin4", [N, 256], bf16, kind="ExternalInput")
    wln = nc.dram_tensor("wln", [4 * D], bf16, kind="ExternalInput")
    bln = nc.dram_tensor("bln", [4 * D], bf16, kind="ExternalInput")
    pb = nc.dram_tensor("pb", [C], f32, kind="ExternalInput")
    out = nc.dram_tensor("out", [TSLICE, C], f32, kind="ExternalOutput")

    NTB = N // 128          # 16 token tiles per batch
    NCT = C // 128          # 8 contraction tiles
    QKW = 3 * HPC * D       # 384

    with tile.TileContext(nc) as tc:
        import contextlib

        stack = contextlib.ExitStack()
        with stack:
            consts = stack.enter_context(tc.tile_pool(name="consts", bufs=1))
            dram = stack.enter_context(tc.tile_pool(name="dram", bufs=2, space="DRAM"))
            persist = stack.enter_context(tc.tile_pool(name="persist", bufs=1))

            # ---- constants (ordered so the first matmul's inputs land first) ----
            wqk_sb = consts.tile([128, NCT, QKW], bf16)
            nc.sync.dma_start(out=wqk_sb, in_=wqk.rearrange("(ct p) f -> p ct f", p=128))
            ident = consts.tile([128, 128], bf16)
            make_identity(nc, ident)
            eps_sb = consts.tile([128, 1], f32)
            nc.vector.memset(eps_sb, EPS)

            # warm-up collective: absorbs the one-time cc-stream setup cost
            # (~10us trigger delay + slow first transfer) during W1 compute
            wu_in = dram.tile([NCORES, 64], bf16, tag="wuin", bufs=1)
            wu_out = dram.tile([NCORES, 64], bf16, tag="wuout", bufs=1)
            nc.sync.dma_start(out=wu_in, in_=cos4[0:2, :])
            nc.gpsimd.collective_compute(
                "AllToAll", mybir.AluOpType.bypass,
                replica_groups=[list(range(NCORES))],
                ins=[wu_in.opt()], outs=[wu_out.opt()])

            # ---- persistent per-batch tensors ----
            qT = [persist.tile([128, N], bf16, tag=f"qT{b}", name=f"qT{b}") for b in range(B)]
            kT2 = [persist.tile([128, N], bf16, tag=f"kT2{b}", name=f"kT2{b}") for b in range(B)]
            # PV stationary: per (tt, h): cols 0-63 = v, col 64 = ones (denominator row)
            vT2 = [persist.tile([128, NTB, HPC, 65], bf16, tag=f"vT2{b}", name=f"vT2{b}")
                   for b in range(B)]
            for b in range(B):
                nc.vector.memset(vT2[b][:, :, :, 64:65], 1.0)
            o_sb = persist.tile([128, T], bf16)  # attn out, channel-major
            stg = [persist.tile([128, NTB, 256], bf16, tag=f"stg{b}", name=f"stg{b}")
                   for b in range(B)]
            # per (tile, pair): bn_stats 6-tuple; q/k weight cols are interleaved
            # host-side so even-stats = head0 of the pair, odd-stats = head1
            st6 = [persist.tile([128, NTB, 2, 6], f32, tag=f"st6{b}", name=f"st6{b}")
                   for b in range(B)]
            rstd = [persist.tile([128, NTB, 4], f32, tag=f"rstd{b}", name=f"rstd{b}")
                    for b in range(B)]
            nmr = [persist.tile([128, NTB, 4], f32, tag=f"nmr{b}", name=f"nmr{b}")
                   for b in range(B)]

            # rope tables (needed from W3 on; DMAs issued after W1's first x tiles)
            cs_sb = consts.tile([128, NTB, 256], bf16)
            sn_sb = consts.tile([128, NTB, 256], bf16)
            if not skip_affine:
                wln_sb = consts.tile([128, 256], bf16)
                bln_sb = consts.tile([128, 256], bf16)
                nc.gpsimd.dma_start(out=wln_sb, in_=bass.AP(tensor=wln, offset=0, ap=[[0, 128], [1, 256]]))
                nc.gpsimd.dma_start(out=bln_sb, in_=bass.AP(tensor=bln, offset=0, ap=[[0, 128], [1, 256]]))

            # a2a buffers, one pair per (batch, icp half). Collective q covers
            # tokens [1024*(q%2), ...) of batch q//2; dest core m receives its
            # 128-token block [128m ..) of that kilobyte range from every core.
            a2a_in = [dram.tile([NCORES, 128, 128], bf16, tag=f"a2ain{q}", bufs=1,
                                name=f"a2ain{q}") for q in range(4)]
            a2a_out = [dram.tile([NCORES, 128, 128], bf16, tag=f"a2aout{q}", bufs=1,
                                 name=f"a2aout{q}") for q in range(4)]

            s1 = stack.enter_context(tc.tile_pool(name="s1", bufs=4))
            xtp = stack.enter_context(tc.tile_pool(name="xt", bufs=2))

            # scope 1 (stage 1, both batches): deep qkv PSUM + transpose slots
            sc1 = contextlib.ExitStack()
            ps_q = sc1.enter_context(tc.tile_pool(name="ps_q", bufs=6, space="PSUM"))
            ps_t1 = sc1.enter_context(tc.tile_pool(name="ps_t1", bufs=2, space="PSUM"))

            def s1A(b, gg, ps_qkv):
                """qkv matmul + v-stash + fused layernorm for 4 token tiles."""
                qps = []
                for half in range(2):
                    col0 = b * N + gg * 512 + half * 256
                    xt = xtp.tile([128, NCT, 256], bf16, tag="xt")
                    nc.sync.dma_start(
                        out=xt,
                        in_=xT.rearrange("(ct p) t -> p ct t", p=128)[:, :, col0:col0 + 256])
                    for sub in range(2):
                        tt = gg * 4 + half * 2 + sub
                        qkv_ps = ps_qkv.tile([128, QKW], f32, tag="qkv", name="qkv_ps")
                        for ct in range(NCT):
                            nc.tensor.matmul(
                                qkv_ps,
                                xt[:, ct, sub * 128:(sub + 1) * 128],
                                wqk_sb[:, ct],
                                start=(ct == 0), stop=(ct == NCT - 1))
                        nc.vector.tensor_copy(
                            out=vT2[b][:, tt, :, 0:64],
                            in_=qkv_ps[:, 256:384].rearrange("p (h d) -> p h d", h=2))
                        for pair in range(2):
                            nc.vector.bn_stats(
                                out=st6[b][:, tt, pair],
                                in_=qkv_ps[:, pair * 128:(pair + 1) * 128])
                        qps.append((tt, qkv_ps))
                # even/odd stats = per-head stats directly (cols interleaved);
                # 6-tuple = [32, mean_h0, 64*var_h0, 32, mean_h1, 64*var_h1]... actually
                # [count_e, mean_e, count_e*var_e, count_o, mean_o, count_o*var_o]
                s6 = st6[b][:, gg * 4:(gg + 1) * 4].rearrange(
                    "p t pr (two three) -> p t pr two three", two=2)
                var4 = s6[:, :, :, :, 2]     # [128, 4, 2, 2] = 64*var per (tile, g)
                mean4 = s6[:, :, :, :, 1]    # [128, 4, 2, 2] = mean per (tile, g)
                rs = rstd[b][:, gg * 4:(gg + 1) * 4]
                rs4 = rs.rearrange("p t (pr two) -> p t pr two", pr=2)
                nm4 = nmr[b][:, gg * 4:(gg + 1) * 4].rearrange(
                    "p t (pr two) -> p t pr two", pr=2)
                nc.scalar.activation(out=rs4, in_=var4, func=AF.Sqrt,
                                     bias=eps_sb, scale=1.0 / 64.0)
                nc.vector.reciprocal_approx_fast(out=rs, in_=rs)
                nc.vector.scalar_tensor_tensor(
                    out=nm4, in0=mean4, scalar=-1.0,
                    in1=rs4, op0=ALU.mult, op1=ALU.mult)
                # normalize straight out of PSUM into bf16 staging (de-interleaving)
                for tt, qkv_ps in qps:
                    qk4 = qkv_ps[:, 0:256].rearrange(
                        "p (pr d two) -> p pr two d", pr=2, two=2)
                    for g in range(4):
                        nc.vector.tensor_scalar(
                            out=stg[b][:, tt, g * 64:(g + 1) * 64],
                            in0=qk4[:, g // 2, g % 2],
                            scalar1=rstd[b][:, tt, g:g + 1],
                            scalar2=nmr[b][:, tt, g:g + 1],
                            op0=ALU.mult, op1=ALU.add)

            def s1B(b, gg, ps_tp, q_on_act=False):
                """rope + transpose for 4 (already normalized) token tiles."""
                tpq = ps_tp.tile([128, 512], bf16, tag="tp", name="tpq")
                tpk = ps_tp.tile([128, 512], bf16, tag="tp", name="tpk")
                for sub in range(4):
                    tt = gg * 4 + sub
                    st = stg[b][:, tt]
                    if not skip_affine:
                        nc.vector.tensor_mul(st, st, wln_sb)
                        nc.vector.tensor_add(st, st, bln_sb)
                    # rope: rq = st*cos + swap(st)*sin   (swap = exchange 32-halves)
                    st4 = st.rearrange("p (g two s) -> p g two s", g=4, two=2)
                    sn4t = sn_sb[:, tt].rearrange("p (g two s) -> p g two s", g=4, two=2)
                    xsw = s1.tile([128, 256], bf16, tag="xsw", bufs=2)
                    xsw4 = xsw.rearrange("p (g two s) -> p g two s", g=4, two=2)
                    nc.gpsimd.tensor_mul(xsw4[:, :, 0, :], st4[:, :, 1, :], sn4t[:, :, 0, :])
                    nc.gpsimd.tensor_mul(xsw4[:, :, 1, :], st4[:, :, 0, :], sn4t[:, :, 1, :])
                    rq = s1.tile([128, 256], bf16, tag="rq", bufs=2)
                    nc.vector.tensor_mul(rq, st, cs_sb[:, tt])
                    nc.vector.tensor_add(rq, rq, xsw)
                    nc.tensor.transpose(tpq[:, sub * 128:(sub + 1) * 128], rq[:, 0:128], ident)
                    nc.tensor.transpose(tpk[:, sub * 128:(sub + 1) * 128], rq[:, 128:256], ident)
                if q_on_act:
                    nc.scalar.activation(out=qT[b][:, gg * 512:(gg + 1) * 512], in_=tpq,
                                         func=AF.Copy)
                else:
                    nc.vector.tensor_copy(out=qT[b][:, gg * 512:(gg + 1) * 512], in_=tpq)
                nc.vector.tensor_copy(out=kT2[b][:, gg * 512:(gg + 1) * 512], in_=tpk)

            def s2_unit(b, h, icp, ps_st, ps_ot):
                """attention for one head, one pair of 512-col i-chunks.
                jt-outer so kT2/vT2 stationary tiles are reused across the pair;
                PV lags one jt behind ST so the PE never stalls on exp."""
                hp = h * 64
                ics = (2 * icp, 2 * icp + 1)
                ot_ps = {ic: ps_ot.tile([128, 512], f32, tag="ot", name=f"ot{b}{h}{ic}")
                         for ic in ics}
                pts = {}
                for jp in range(NTB // 2 + 1):
                    if jp < NTB // 2:
                        for ic in ics:
                            st_ps = ps_st.tile([128, 1024], f32, tag="st")
                            for half in range(2):
                                jt = 2 * jp + half
                                nc.tensor.matmul(
                                    st_ps[:, half * 512:(half + 1) * 512],
                                    kT2[b][hp:hp + 64, jt * 128:(jt + 1) * 128],
                                    qT[b][hp:hp + 64, ic * 512:(ic + 1) * 512],
                                    start=True, stop=True)
                            p_t = s1.tile([128, 1024], bf16, tag="pt")
                            nc.scalar.activation(out=p_t, in_=st_ps, func=AF.Exp,
                                                 scale=SCALE)
                            pts[(jp, ic)] = p_t
                    if jp > 0:
                        for ic in ics:
                            p_t = pts.pop((jp - 1, ic))
                            for half in range(2):
                                jt = 2 * (jp - 1) + half
                                nc.tensor.matmul(
                                    ot_ps[ic][0:65, :],
                                    vT2[b][:, jt, h, :],
                                    p_t[:, half * 512:(half + 1) * 512],
                                    start=(jp == 1 and half == 0),
                                    stop=(jp == NTB // 2 and half == 1))
                for ic in ics:
                    rd = s1.tile([1, 512], f32, tag="rd", bufs=2)
                    nc.vector.tensor_copy(out=rd, in_=ot_ps[ic][64:65, :])
                    scr = dram.tile([512], f32, tag="scr")
                    nc.sync.dma_start(out=scr, in_=rd)
                    bc = s1.tile([64, 512], f32, tag="bc", bufs=2)
                    nc.gpsimd.dma_start(
                        out=bc,
                        in_=bass.AP(tensor=scr.tensor, offset=scr.offset,
                                    ap=[[0, 64]] + [list(x) for x in scr.ap]))
                    nc.vector.reciprocal_approx_fast(out=bc, in_=bc)
                    nc.vector.tensor_mul(
                        o_sb[hp:hp + 64, b * N + ic * 512:b * N + (ic + 1) * 512],
                        ot_ps[ic][0:64, :], bc)

            def stage_a2a(b, icp):
                """after both heads of (b, icp) are done, ship those token
                blocks to the all-to-all input buffer and start the exchange."""
                q = 2 * b + icp
                col0 = b * N + icp * 1024
                for m in range(NCORES):
                    nc.sync.dma_start(
                        out=a2a_in[q][m],
                        in_=o_sb[:, col0 + m * 128:col0 + (m + 1) * 128])

            def run_a2a(q):
                nc.gpsimd.collective_compute(
                    "AllToAll",
                    mybir.AluOpType.bypass,
                    replica_groups=[list(range(NCORES))],
                    ins=[a2a_in[q].opt()],
                    outs=[a2a_out[q].opt()],
                )

            # projection weights/bias (DMAs issued mid-W3 in the schedule below)
            pwT_sb = consts.tile([128, NCT, C], bf16)
            pb_sb = consts.tile([128, C], f32)

            def proj(q, ps_pj):
                """output projection for one 128-token tile (collective q)."""
                otf = s1.tile([128, NCT, 128], bf16, tag=f"otf{q}", bufs=1,
                              name=f"otf{q}")
                # ACT-issued so the cc-gated wait never sits at the head of the
                # sync queue (whose cumulative DMA semaphores gate PE ldweights)
                for ct in range(NCT):
                    nc.scalar.dma_start(out=otf[:, ct], in_=a2a_out[q][ct])
                pp = {}
                for oc in range(2):
                    pp[oc] = ps_pj.tile([128, 512], f32, tag="tp", name=f"pj{q}{oc}")
                for ct in range(NCT):
                    for oc in range(2):
                        nc.tensor.matmul(
                            pp[oc],
                            otf[:, ct],
                            pwT_sb[:, ct, oc * 512:(oc + 1) * 512],
                            start=(ct == 0), stop=(ct == NCT - 1))
                for oc in range(2):
                    o_st = s1.tile([128, 512], f32, tag="ost", bufs=2)
                    nc.vector.tensor_add(o_st, pp[oc], pb_sb[:, oc * 512:(oc + 1) * 512])
                    nc.sync.dma_start(
                        out=out[q * 128:(q + 1) * 128, oc * 512:(oc + 1) * 512],
                        in_=o_st)

            # ---------------- emission schedule ----------------
            for gg in range(4):             # W1: qkv+LN batch 0
                s1A(0, gg, ps_q)
                if gg == 0:                 # rope tables: after the first x tiles
                    nc.sync.dma_start(out=cs_sb,
                                      in_=cos4.rearrange("(t p) c -> p t c", p=128))
                    nc.sync.dma_start(out=sn_sb,
                                      in_=sin4.rearrange("(t p) c -> p t c", p=128))
            for gg in range(4):             # W3: rope b0 (DVE/Pool) || qkv+LN b1 (PE/ACT)
                s1B(0, gg, ps_t1, q_on_act=True)
                s1A(1, gg, ps_q)
                if gg == 0:                 # projection consts: mid-kernel
                    nc.sync.dma_start(out=pwT_sb,
                                      in_=pwT.rearrange("(ct p) o -> p ct o", p=128))
                    nc.gpsimd.dma_start(out=pb_sb, in_=bass.AP(
                        tensor=pb, offset=0, ap=[[0, 128], [1, C]]))
            sc1.close()
            # scope 2 (attention + projection)
            sc2 = contextlib.ExitStack()
            ps_st = sc2.enter_context(tc.tile_pool(name="ps_st", bufs=2, space="PSUM"))
            ps_ot = sc2.enter_context(tc.tile_pool(name="ps_ot", bufs=2, space="PSUM"))
            ps_b = sc2.enter_context(tc.tile_pool(name="ps_b", bufs=2, space="PSUM"))
            with sc2:
                for u in range(4):          # W5: attention b0 || rope b1
                    s1B(1, u, ps_b)
                    s2_unit(0, u % 2, u // 2, ps_st, ps_ot)
                    if u % 2 == 1:
                        stage_a2a(0, u // 2)
                    if u == 1:
                        run_a2a(0)
                for u in range(4):          # W6: attention b1 || a2a+proj b0
                    s2_unit(1, u % 2, u // 2, ps_st, ps_ot)
                    if u == 0:
                        run_a2a(1)
                    if u % 2 == 1:
                        stage_a2a(1, u // 2)
                    if u == 2:
                        run_a2a(2)
                    if u == 3:
                        proj(0, ps_b)
                run_a2a(3)                  # W7: tail
                proj(1, ps_b)
                proj(2, ps_b)
                proj(3, ps_b)

    nc.finalize()
    _BUILT[key] = nc
    return nc


def _host_prep(x, qkv_w, qn_w, qn_b, kn_w, kn_b, proj_w, proj_b, pos):
    bf16 = ml_dtypes.bfloat16
    x = np.asarray(x, dtype=np.float32)
    qkv_w = np.asarray(qkv_w, dtype=np.float32)
    proj_w = np.asarray(proj_w, dtype=np.float32)
    pos = np.asarray(pos)

    xT = np.ascontiguousarray(x.reshape(T, C).T.astype(bf16))
    pwT = np.ascontiguousarray(proj_w.T.astype(bf16))

    d2 = D // 2
    inv_freq = (np.float32(1.0) / (np.float32(10000.0) **
                (np.arange(d2, dtype=np.float32) / np.float32(d2)))).astype(np.float32)
    ang = pos.astype(np.float32)[:, None] * inv_freq[None, :]
    cos = np.cos(ang).astype(np.float32)
    sin = np.sin(ang).astype(np.float32)
    cos4 = np.ascontiguousarray(np.tile(cos, (1, 8)).astype(bf16))
    sin4 = np.ascontiguousarray(
        np.tile(np.concatenate([-sin, sin], axis=1), (1, 4)).astype(bf16))

    wln = np.ascontiguousarray(np.concatenate(
        [qn_w, qn_w, kn_w, kn_w]).astype(np.float32))
    bln = np.ascontiguousarray(np.concatenate(
        [qn_b, qn_b, kn_b, kn_b]).astype(np.float32))
    pb = np.ascontiguousarray(np.asarray(proj_b, dtype=np.float32))
    skip_affine = bool(np.all(wln == 1.0) and np.all(bln == 0.0))

    # interleave the two heads within the q block and within the k block so
    # bn_stats' even/odd split yields per-head stats directly
    qk_perm = np.arange(256).reshape(2, 2, 64).transpose(0, 2, 1).ravel()
    in_maps = []
    for k in range(NCORES):
        rows = slice(128 * k, 128 * (k + 1))
        wqk_k = np.concatenate(
            [qkv_w[rows], qkv_w[C:][rows], qkv_w[2 * C:][rows]], axis=0).T
        wqk_k[:, 0:256] = wqk_k[:, qk_perm]
        wqk_k = np.ascontiguousarray(wqk_k.astype(bf16))
        in_maps.append({
            "xT": xT, "wqk": wqk_k, "pwT": pwT,
            "cos4": cos4, "sin4": sin4,
            "wln": wln.astype(bf16), "bln": bln.astype(bf16), "pb": pb,
        })
    return in_maps, skip_affine


def run_on_device(inputs, trace=False):
    from concourse.bass_utils import run_bass_kernel_spmd

    in_maps, skip_affine = _host_prep(**inputs)
    nc = _build(skip_affine)
    res = run_bass_kernel_spmd(nc, in_maps, list(range(NCORES)), trace=trace)
    # core m's 4 row-tiles q=0..3 hold batch q//2 tokens [1024*(q%2)+128m ..)
    out_full = np.empty((B, N, C), dtype=np.float32)
    for m in range(NCORES):
        r = res.results[m]["out"]
        for q in range(4):
            t0 = 1024 * (q % 2) + 128 * m
            out_full[q // 2, t0:t0 + 128] = r[q * 128:(q + 1) * 128]
    return out_full, res


def kernel(**inputs):
    out, _ = run_on_device(inputs, trace=False)
    return out
